# revision 5
# baseline (speedup 1.0000x reference)
"""Local (windowed causal) attention with RoPE — Trainium2 Bass kernel.

Problem: B=4, H=16, T=4096, E=64, WINDOW=128, look_backward=1, causal.
Sharding: merged batch*heads (64 rows) split 8 per NeuronCore across 8 cores.

Per (bh, window w of 128 queries): keys/values = windows {w-1, w}.
On-chip pipeline per bh:
  - DMA q/k/v in "(c)-layout" [128, nw*64]: tile[p, w*64+e] = x[w*128+p, e]
  - RoPE on DVE (whole-bh batched tensor_tensor ops)
  - per window: PE transpose q',k' -> qT/kT [64, T]; scores transposed
    st[j, i] via matmul(lhsT=kT_w, rhs=[qT_w | qT_{w+1}]) (one N=256 matmul
    produces both the "cur" block for w and the "prev" block for w+1);
    ACT exp(scale=1/8); DVE multiplicative causal mask on the cur half;
    PV via matmul(lhsT=E_block, rhs=[v|1]) accumulating out and the softmax
    denominator in one PSUM tile [128, 65]; DVE reciprocal; ACT copy*scale;
    DMA out.
"""

from contextlib import ExitStack

import numpy as np

import concourse.bass as bass
import concourse.bacc as bacc
import concourse.mybir as mybir
from concourse import tile
from concourse import bass_utils

F32 = mybir.dt.float32
E = 64
W = 128
HALF = 32
B, H, T = 4, 16, 4096
N_CORES = 8
N_BH = (B * H) // N_CORES  # bh rows per core
NW = T // W


# ---------------------------------------------------------------- host consts
def _rope_tables():
    nw = T // W
    inv_freq = 1.0 / (10000.0 ** (np.arange(0, E, 2, dtype=np.float32) / E))
    t = np.arange(T, dtype=np.float32)
    freqs = np.outer(t, inv_freq)
    emb = np.concatenate([freqs, freqs], axis=-1)
    cos = np.cos(emb).astype(np.float32)
    sin = np.sin(emb).astype(np.float32)
    sinA = np.concatenate([-sin[:, :HALF], sin[:, HALF:]], axis=-1).astype(np.float32)

    def to_c(x):
        return np.ascontiguousarray(
            x.reshape(nw, W, E).transpose(1, 0, 2).reshape(W, nw * E)
        )

    return to_c(cos), to_c(sinA)


def _mask01():
    j = np.arange(W)[:, None]
    i = np.arange(W)[None, :]
    return (i >= j).astype(np.float32)


# ---------------------------------------------------------------- device body
def _body(ctx, tc, out_ap, q_ap, k_ap, v_ap, cos_ap, sinA_ap, mask_ap, ident_ap):
    nc = tc.nc
    n_bh = q_ap.shape[0]
    FB = NW * E

    const = ctx.enter_context(tc.tile_pool(name="const", bufs=1))
    big = ctx.enter_context(tc.tile_pool(name="big", bufs=2))
    tbuf = ctx.enter_context(tc.tile_pool(name="tbuf", bufs=1))
    ering = ctx.enter_context(tc.tile_pool(name="ering", bufs=4))
    small = ctx.enter_context(tc.tile_pool(name="small", bufs=4))
    pt_q = ctx.enter_context(tc.tile_pool(name="pt_q", bufs=2, space="PSUM"))
    pt_k = ctx.enter_context(tc.tile_pool(name="pt_k", bufs=2, space="PSUM"))
    pst = ctx.enter_context(tc.tile_pool(name="pst", bufs=2, space="PSUM"))
    pov = ctx.enter_context(tc.tile_pool(name="pov", bufs=2, space="PSUM"))

    cos_c = const.tile([128, FB], F32)
    nc.sync.dma_start(cos_c[:, :], cos_ap)
    sinA_c = const.tile([128, FB], F32)
    nc.sync.dma_start(sinA_c[:, :], sinA_ap)
    mask_c = const.tile([128, 128], F32)
    nc.sync.dma_start(mask_c[:, :], mask_ap)
    ident_c = const.tile([128, 128], F32)
    nc.sync.dma_start(ident_c[:, :], ident_ap)

    for bh in range(n_bh):
        qn = big.tile([128, FB], F32)
        nc.sync.dma_start(
            qn.rearrange("p (n e) -> p n e", e=E),
            q_ap[bh].rearrange("(n p) e -> p n e", p=128),
        )
        kn = big.tile([128, FB], F32)
        nc.sync.dma_start(
            kn.rearrange("p (n e) -> p n e", e=E),
            k_ap[bh].rearrange("(n p) e -> p n e", p=128),
        )
        vx = big.tile([128, NW * (E + 1)], F32)
        vx3 = vx.rearrange("p (n c) -> p n c", c=E + 1)
        nc.sync.dma_start(
            vx3[:, :, 0:E], v_ap[bh].rearrange("(n p) e -> p n e", p=128)
        )
        nc.vector.memset(vx3[:, :, E : E + 1], 1.0)

        def rope(xn, name):
            t2 = big.tile([128, FB], F32, name=f"t2_{name}")
            x4 = xn.rearrange("p (n two h) -> p n two h", two=2, h=HALF)
            t4 = t2.rearrange("p (n two h) -> p n two h", two=2, h=HALF)
            s4 = sinA_c.rearrange("p (n two h) -> p n two h", two=2, h=HALF)
            nc.vector.tensor_mul(t4[:, :, 0, :], x4[:, :, 1, :], s4[:, :, 0, :])
            nc.vector.tensor_mul(t4[:, :, 1, :], x4[:, :, 0, :], s4[:, :, 1, :])
            xc = tbuf.tile([128, FB], F32, name=f"xc_{name}", tag="xc")
            nc.vector.tensor_mul(xc[:, :], xn[:, :], cos_c[:, :])
            nc.vector.tensor_add(t2[:, :], xc[:, :], t2[:, :])
            return t2

        qr = rope(qn, "q")
        kr = rope(kn, "k")

        qT = tbuf.tile([64, T], F32)
        kT = tbuf.tile([64, T], F32)
        for w in range(NW):
            ptq = pt_q.tile([64, 128], F32)
            nc.tensor.matmul(
                ptq[:, :], qr[:, w * E : (w + 1) * E], ident_c[:, :],
                is_transpose=True,
            )
            nc.scalar.copy(qT[:, w * W : (w + 1) * W], ptq[:, :])
            ptk = pt_k.tile([64, 128], F32)
            nc.tensor.matmul(
                ptk[:, :], kr[:, w * E : (w + 1) * E], ident_c[:, :],
                is_transpose=True,
            )
            nc.scalar.copy(kT[:, w * W : (w + 1) * W], ptk[:, :])

        e_tiles = [None] * NW
        for w in range(NW):
            n_cols = 256 if w + 1 < NW else 128
            st = pst.tile([128, 256], F32)
            nc.tensor.matmul(
                st[:, 0:n_cols],
                kT[:, w * W : (w + 1) * W],
                qT[:, w * W : w * W + n_cols],
            )
            et = ering.tile([128, 256], F32)
            nc.scalar.activation(
                et[:, 0:n_cols], st[:, 0:n_cols],
                mybir.ActivationFunctionType.Exp, scale=float(E) ** -0.5,
            )
            nc.vector.tensor_mul(et[:, 0:128], et[:, 0:128], mask_c[:, :])
            e_tiles[w] = et

            ov = pov.tile([128, E + 1], F32)
            if w == 0:
                nc.tensor.matmul(
                    ov[:, :], et[:, 0:128], vx3[:, w, :], start=True, stop=True
                )
            else:
                ep = e_tiles[w - 1]
                nc.tensor.matmul(
                    ov[:, :], ep[:, 128:256], vx3[:, w - 1, :], start=True, stop=False
                )
                nc.tensor.matmul(
                    ov[:, :], et[:, 0:128], vx3[:, w, :], start=False, stop=True
                )
            r = small.tile([128, 1], F32)
            nc.vector.reciprocal(r[:, :], ov[:, E : E + 1])
            ow = small.tile([128, E], F32)
            nc.scalar.activation(
                ow[:, :], ov[:, 0:E],
                mybir.ActivationFunctionType.Copy, scale=r[:, 0:1],
            )
            nc.sync.dma_start(out_ap[bh, w * W : (w + 1) * W, :], ow[:, :])


# ---------------------------------------------------------------- build & run
_CACHE = {}


def _build():
    if "nc" in _CACHE:
        return _CACHE["nc"]
    nc = bacc.Bacc(
        "TRN2",
        target_bir_lowering=False,
        debug=False,
        enable_asserts=True,
        num_devices=N_CORES,
    )
    shp = [N_BH, T, E]
    q = nc.dram_tensor("q", shp, F32, kind="ExternalInput").ap()
    k = nc.dram_tensor("k", shp, F32, kind="ExternalInput").ap()
    v = nc.dram_tensor("v", shp, F32, kind="ExternalInput").ap()
    cos = nc.dram_tensor("cos", [128, NW * E], F32, kind="ExternalInput").ap()
    sinA = nc.dram_tensor("sinA", [128, NW * E], F32, kind="ExternalInput").ap()
    mask = nc.dram_tensor("mask", [128, 128], F32, kind="ExternalInput").ap()
    ident = nc.dram_tensor("ident", [128, 128], F32, kind="ExternalInput").ap()
    out = nc.dram_tensor("out", shp, F32, kind="ExternalOutput").ap()
    with tile.TileContext(nc) as tc:
        with ExitStack() as ctx:
            _body(ctx, tc, out, q, k, v, cos, sinA, mask, ident)
    nc.finalize()
    _CACHE["nc"] = nc
    return nc


def kernel(q, k, v, trace=False):
    nc = _build()
    cos_c, sinA_c = _rope_tables()
    mask = _mask01()
    ident = np.eye(128, dtype=np.float32)

    qm = np.ascontiguousarray(q.reshape(B * H, T, E))
    km = np.ascontiguousarray(k.reshape(B * H, T, E))
    vm = np.ascontiguousarray(v.reshape(B * H, T, E))
    in_maps = []
    for c in range(N_CORES):
        s = slice(c * N_BH, (c + 1) * N_BH)
        in_maps.append(
            {
                "q": np.ascontiguousarray(qm[s]),
                "k": np.ascontiguousarray(km[s]),
                "v": np.ascontiguousarray(vm[s]),
                "cos": cos_c,
                "sinA": sinA_c,
                "mask": mask,
                "ident": ident,
            }
        )
    res = bass_utils.run_bass_kernel_spmd(
        nc, in_maps, core_ids=list(range(N_CORES)), trace=trace
    )
    out = np.concatenate([r["out"] for r in res.results], axis=0)
    out = out.reshape(q.shape).astype(np.float32)
    if trace:
        return out, res
    return out


# revision 8
# speedup vs baseline: 2.5722x; 2.5722x over previous
"""Local (windowed causal) attention with RoPE — Trainium2 Bass kernel.

Problem: B=4, H=16, T=4096, E=64, WINDOW=128, look_backward=1, causal.
Sharding: merged batch*heads (64 rows) split 8 per NeuronCore across 8 cores.

v2 design:
- q/k/v DMA'd per bh via SWDGE (gpsimd) in "(c)-layout" [128, nw*64]
  (tile[p, w*64+e] = x[w*128+p, e]); v cast to bf16 during DMA.
- RoPE in fp32 on DVE (whole-bh ops), final add writes bf16.
- PE transposes (bf16, 4-window batches) -> qT/kT [64, T] bf16
  (copies: qT on ACT, kT on DVE).
- Scores transposed, window pairs: st[j,i] f32 PSUM [128, 512] via
  matmul(lhsT=kT_w [64,128], rhs=qT[w..w+2) [64,256]) per window; one
  ACT exp (scale=8^-1) -> E bf16; DVE causal mask (mul by 0/1) on cur halves.
- PV: matmul(lhsT=E block bf16 [128,128], rhs=[v|1] bf16 [128,65]) accumulating
  out + denominator in PSUM [128, 260] (4 windows); batched DVE reciprocal and
  broadcast-multiply normalize; one store per 4 windows.
"""

from contextlib import ExitStack

import numpy as np

import concourse.bass as bass
import concourse.bacc as bacc
import concourse.mybir as mybir
from concourse import tile
from concourse import bass_utils

F32 = mybir.dt.float32
BF16 = mybir.dt.bfloat16
E = 64
W = 128
HALF = 32
B, H, T = 4, 16, 4096
N_CORES = 8
N_BH = (B * H) // N_CORES
NW = T // W
WT = 4  # windows per transpose/psum batch


# ---------------------------------------------------------------- host consts
def _rope_tables(t_len=T):
    nw = t_len // W
    inv_freq = 1.0 / (10000.0 ** (np.arange(0, E, 2, dtype=np.float32) / E))
    t = np.arange(t_len, dtype=np.float32)
    freqs = np.outer(t, inv_freq)
    emb = np.concatenate([freqs, freqs], axis=-1)
    cos = np.cos(emb).astype(np.float32)
    sin = np.sin(emb).astype(np.float32)
    sinA = np.concatenate([-sin[:, :HALF], sin[:, HALF:]], axis=-1).astype(np.float32)

    def to_c(x):
        return np.ascontiguousarray(
            x.reshape(nw, W, E).transpose(1, 0, 2).reshape(W, nw * E)
        )

    return to_c(cos), to_c(sinA)


def _mask01():
    j = np.arange(W)[:, None]
    i = np.arange(W)[None, :]
    m = (i >= j).astype(np.float32)
    return np.concatenate([m, m], axis=1)  # [128, 256] two copies


# ---------------------------------------------------------------- device body
def _body(ctx, tc, out_ap, q_ap, k_ap, v_ap, cos_ap, sinA_ap, mask_ap, ident_ap,
          n_bh, nw):
    nc = tc.nc
    FB = nw * E
    t_len = nw * W
    n_pair = nw // 2

    const = ctx.enter_context(tc.tile_pool(name="const", bufs=1))
    big = ctx.enter_context(tc.tile_pool(name="big", bufs=2))
    tbuf = ctx.enter_context(tc.tile_pool(name="tbuf", bufs=1))
    ering = ctx.enter_context(tc.tile_pool(name="ering", bufs=3))
    oring = ctx.enter_context(tc.tile_pool(name="oring", bufs=3))
    small = ctx.enter_context(tc.tile_pool(name="small", bufs=4))
    pt_q = ctx.enter_context(tc.tile_pool(name="pt_q", bufs=2, space="PSUM"))
    pt_k = ctx.enter_context(tc.tile_pool(name="pt_k", bufs=2, space="PSUM"))
    pst = ctx.enter_context(tc.tile_pool(name="pst", bufs=2, space="PSUM"))
    pov = ctx.enter_context(tc.tile_pool(name="pov", bufs=2, space="PSUM"))

    cos_c = const.tile([128, FB], F32)
    nc.sync.dma_start(cos_c[:, :], cos_ap)
    sinA_c = const.tile([128, FB], F32)
    nc.sync.dma_start(sinA_c[:, :], sinA_ap)
    mask_c = const.tile([128, 256], BF16)
    nc.gpsimd.dma_start(mask_c[:, :], mask_ap)  # casts f32 -> bf16
    ident_c = const.tile([128, 128], BF16)
    nc.gpsimd.dma_start(ident_c[:, :], ident_ap)

    for bh in range(n_bh):
        qn = big.tile([128, FB], F32)
        nc.gpsimd.dma_start(
            qn.rearrange("p (n e) -> p n e", e=E),
            q_ap[bh].rearrange("(n p) e -> p n e", p=128),
        )
        kn = big.tile([128, FB], F32)
        nc.gpsimd.dma_start(
            kn.rearrange("p (n e) -> p n e", e=E),
            k_ap[bh].rearrange("(n p) e -> p n e", p=128),
        )
        vx = big.tile([128, nw * (E + 1)], BF16)
        vx3 = vx.rearrange("p (n c) -> p n c", c=E + 1)
        nc.gpsimd.dma_start(
            vx3[:, :, 0:E], v_ap[bh].rearrange("(n p) e -> p n e", p=128)
        )
        nc.gpsimd.memset(vx3[:, :, E : E + 1], 1.0)

        # rope: x' = x*cos + swapped(x)*sinA; final add writes bf16
        def rope(xn, name):
            t2 = big.tile([128, FB], F32, name=f"t2_{name}")
            xb = big.tile([128, FB], BF16, name=f"xb_{name}")
            x4 = xn.rearrange("p (n two h) -> p n two h", two=2, h=HALF)
            t4 = t2.rearrange("p (n two h) -> p n two h", two=2, h=HALF)
            s4 = sinA_c.rearrange("p (n two h) -> p n two h", two=2, h=HALF)
            nc.vector.tensor_mul(t4[:, :, 0, :], x4[:, :, 1, :], s4[:, :, 0, :])
            nc.vector.tensor_mul(t4[:, :, 1, :], x4[:, :, 0, :], s4[:, :, 1, :])
            xc = tbuf.tile([128, FB], F32, name=f"xc_{name}", tag="xc")
            nc.vector.tensor_mul(xc[:, :], xn[:, :], cos_c[:, :])
            nc.vector.tensor_add(xb[:, :], xc[:, :], t2[:, :])
            return xb

        qrb = rope(qn, "q")
        krb = rope(kn, "k")

        # transposes, WT windows per PSUM batch
        qT = tbuf.tile([64, t_len], BF16)
        kT = tbuf.tile([64, t_len], BF16)
        for g in range(nw // WT):
            ptq = pt_q.tile([64, WT * 128], BF16)
            ptk = pt_k.tile([64, WT * 128], BF16)
            for j in range(WT):
                w = g * WT + j
                nc.tensor.matmul(
                    ptq[:, j * 128 : (j + 1) * 128],
                    qrb[:, w * E : (w + 1) * E], ident_c[:, :],
                    is_transpose=True,
                )
                nc.tensor.matmul(
                    ptk[:, j * 128 : (j + 1) * 128],
                    krb[:, w * E : (w + 1) * E], ident_c[:, :],
                    is_transpose=True,
                )
            sl = slice(g * WT * 128, (g + 1) * WT * 128)
            nc.scalar.copy(qT[:, sl], ptq[:, :])
            nc.vector.tensor_copy(kT[:, sl], ptk[:, :])

        # score pairs / exp / mask / PV / normalize
        e_tiles = [None] * n_pair
        ov_tiles = [None] * (nw // WT)
        for m in range(n_pair):
            w0, w1 = 2 * m, 2 * m + 1
            st = pst.tile([128, 512], F32)
            nc.tensor.matmul(
                st[:, 0:256],
                kT[:, w0 * W : (w0 + 1) * W],
                qT[:, w0 * W : (w0 + 2) * W],
            )
            n1 = 256 if w1 + 1 < nw else 128
            nc.tensor.matmul(
                st[:, 256 : 256 + n1],
                kT[:, w1 * W : (w1 + 1) * W],
                qT[:, w1 * W : w1 * W + n1],
            )
            et = ering.tile([128, 512], BF16)
            nc.scalar.activation(
                et[:, 0 : 256 + n1], st[:, 0 : 256 + n1],
                mybir.ActivationFunctionType.Exp, scale=float(E) ** -0.5,
            )
            cur = et.rearrange("p (two c) -> p two c", two=2)[:, :, 0:128]
            msk = mask_c.rearrange("p (two c) -> p two c", two=2)
            nc.vector.tensor_mul(cur, cur, msk)
            e_tiles[m] = et

            for w in (w0, w1):
                gm = w // WT
                if w % WT == 0:
                    ov_tiles[gm] = pov.tile([128, WT * (E + 1)], F32, name="ov")
                ov = ov_tiles[gm]
                osl = slice((w % WT) * (E + 1), (w % WT + 1) * (E + 1))
                if w == 0:
                    e_cur = e_tiles[0][:, 0:128]
                    nc.tensor.matmul(
                        ov[:, osl], e_cur, vx3[:, 0, :], start=True, stop=True
                    )
                else:
                    if w % 2 == 0:
                        e_prev = e_tiles[m - 1][:, 384:512]
                        e_cur = e_tiles[m][:, 0:128]
                    else:
                        e_prev = e_tiles[m][:, 128:256]
                        e_cur = e_tiles[m][:, 256:384]
                    nc.tensor.matmul(
                        ov[:, osl], e_prev, vx3[:, w - 1, :], start=True, stop=False
                    )
                    nc.tensor.matmul(
                        ov[:, osl], e_cur, vx3[:, w, :], start=False, stop=True
                    )

            # after finishing a WT-group, normalize + store
            if w1 % WT == WT - 1:
                g = w1 // WT
                ov = ov_tiles[g]
                ov3 = ov.rearrange("p (n c) -> p n c", c=E + 1)
                r = small.tile([128, WT], F32)
                nc.vector.reciprocal(r[:, :], ov3[:, :, E])
                ot = oring.tile([128, WT * E], F32)
                rb = r[:, :, None].broadcast_to([128, WT, E])
                nc.vector.tensor_mul(
                    ot.rearrange("p (n e) -> p n e", e=E), ov3[:, :, 0:E], rb
                )
                nc.sync.dma_start(
                    out_ap[bh, g * WT * W : (g + 1) * WT * W, :].rearrange(
                        "(n p) e -> p n e", p=128
                    ),
                    ot.rearrange("p (n e) -> p n e", e=E),
                )


# ---------------------------------------------------------------- build & run
_CACHE = {}


def _build():
    if "nc" in _CACHE:
        return _CACHE["nc"]
    nc = bacc.Bacc(
        "TRN2",
        target_bir_lowering=False,
        debug=False,
        enable_asserts=True,
        num_devices=N_CORES,
    )
    shp = [N_BH, T, E]
    q = nc.dram_tensor("q", shp, F32, kind="ExternalInput").ap()
    k = nc.dram_tensor("k", shp, F32, kind="ExternalInput").ap()
    v = nc.dram_tensor("v", shp, F32, kind="ExternalInput").ap()
    cos = nc.dram_tensor("cos", [128, NW * E], F32, kind="ExternalInput").ap()
    sinA = nc.dram_tensor("sinA", [128, NW * E], F32, kind="ExternalInput").ap()
    mask = nc.dram_tensor("mask", [128, 256], F32, kind="ExternalInput").ap()
    ident = nc.dram_tensor("ident", [128, 128], F32, kind="ExternalInput").ap()
    out = nc.dram_tensor("out", shp, F32, kind="ExternalOutput").ap()
    with tile.TileContext(nc) as tc:
        with ExitStack() as ctx:
            _body(ctx, tc, out, q, k, v, cos, sinA, mask, ident, N_BH, NW)
    nc.finalize()
    _CACHE["nc"] = nc
    return nc


def kernel(q, k, v, trace=False):
    nc = _build()
    cos_c, sinA_c = _rope_tables()
    mask = _mask01()
    ident = np.eye(128, dtype=np.float32)

    qm = np.ascontiguousarray(q.reshape(B * H, T, E))
    km = np.ascontiguousarray(k.reshape(B * H, T, E))
    vm = np.ascontiguousarray(v.reshape(B * H, T, E))
    in_maps = []
    for c in range(N_CORES):
        s = slice(c * N_BH, (c + 1) * N_BH)
        in_maps.append(
            {
                "q": np.ascontiguousarray(qm[s]),
                "k": np.ascontiguousarray(km[s]),
                "v": np.ascontiguousarray(vm[s]),
                "cos": cos_c,
                "sinA": sinA_c,
                "mask": mask,
                "ident": ident,
            }
        )
    res = bass_utils.run_bass_kernel_spmd(
        nc, in_maps, core_ids=list(range(N_CORES)), trace=trace
    )
    out = np.concatenate([r["out"] for r in res.results], axis=0)
    out = out.reshape(q.shape).astype(np.float32)
    if trace:
        return out, res
    return out


# revision 13
# speedup vs baseline: 3.4486x; 1.3407x over previous
"""Local (windowed causal) attention with RoPE — Trainium2 Bass kernel.

Problem: B=4, H=16, T=4096, E=64, WINDOW=128, look_backward=1, causal.
Sharding: merged batch*heads (64 rows) split 8 per NeuronCore across 8 cores.

v3 design (all matmul operands bf16, fp32 accumulation):
- q/k/v loaded per bh via SWDGE (gpsimd) with f32->bf16 cast, "(c)-layout"
  [128, nw*64] (tile[p, w*64+e] = x[w*128+p, e]).
- RoPE in bf16 on DVE (whole-bh ops, 2x mode), host-precomputed bf16 tables.
- PE transposes into one merged PSUM bank [64, 1024] per 4 windows
  (q cols 0:512, k cols 512:1024) -> qT/kT [64, T] bf16; copies on ACT.
- Scores: 4 windows per PSUM tile [128, 1024] f32; st[j,i] via
  matmul(lhsT=kT_w [64,128], rhs=qT[w..w+2) [64,256]); one ACT exp
  (scale=8^-0.5... E**-0.5) -> E bf16 [128,1024]; DVE causal mask on cur
  halves (strided, mul by 0/1 bf16).
- PV: matmul(lhsT=E block [128,128] bf16, rhs=[v|1] bf16 [128,65]) accumulating
  out + softmax denominator in PSUM [128, 260] (4 windows); batched DVE
  reciprocal + broadcast-mul normalize into a per-bh out buffer; one 1MB
  store per bh on the sync ring.
"""

from contextlib import ExitStack

import numpy as np
import ml_dtypes

import concourse.bass as bass
import concourse.bacc as bacc
import concourse.mybir as mybir
from concourse import tile
from concourse import bass_utils

F32 = mybir.dt.float32
BF16 = mybir.dt.bfloat16
NP_BF16 = ml_dtypes.bfloat16
E = 64
W = 128
HALF = 32
B, H, T = 4, 16, 4096
N_CORES = 8
N_BH = (B * H) // N_CORES
NW = T // W
WT = 4  # windows per transpose/score/output batch


# ---------------------------------------------------------------- host consts
def _rope_tables(t_len=T):
    nw = t_len // W
    inv_freq = 1.0 / (10000.0 ** (np.arange(0, E, 2, dtype=np.float32) / E))
    t = np.arange(t_len, dtype=np.float32)
    freqs = np.outer(t, inv_freq)
    emb = np.concatenate([freqs, freqs], axis=-1)
    cos = np.cos(emb).astype(np.float32)
    sin = np.sin(emb).astype(np.float32)
    sinA = np.concatenate([-sin[:, :HALF], sin[:, HALF:]], axis=-1).astype(np.float32)

    def to_c(x):
        return np.ascontiguousarray(
            x.reshape(nw, W, E).transpose(1, 0, 2).reshape(W, nw * E)
        ).astype(NP_BF16)

    return to_c(cos), to_c(sinA)


def _mask01():
    j = np.arange(W)[:, None]
    i = np.arange(W)[None, :]
    m = (i >= j).astype(np.float32)
    return np.concatenate([m, m, m, m], axis=1).astype(NP_BF16)  # [128, 512]


# ---------------------------------------------------------------- device body
def _body(ctx, tc, out_ap, q_ap, k_ap, v_ap, cos_ap, sinA_ap, mask_ap, ident_ap,
          n_bh, nw):
    nc = tc.nc
    FB = nw * E
    t_len = nw * W
    n_grp = nw // WT

    const = ctx.enter_context(tc.tile_pool(name="const", bufs=1))
    big = ctx.enter_context(tc.tile_pool(name="big", bufs=2))
    tbuf = ctx.enter_context(tc.tile_pool(name="tbuf", bufs=1))
    ering = ctx.enter_context(tc.tile_pool(name="ering", bufs=3))
    small = ctx.enter_context(tc.tile_pool(name="small", bufs=4))
    ptp = ctx.enter_context(tc.tile_pool(name="ptp", bufs=2, space="PSUM"))
    pst = ctx.enter_context(tc.tile_pool(name="pst", bufs=2, space="PSUM"))
    pov = ctx.enter_context(tc.tile_pool(name="pov", bufs=2, space="PSUM"))

    cos_c = const.tile([128, FB], BF16)
    nc.sync.dma_start(cos_c[:, :], cos_ap)
    sinA_c = const.tile([128, FB], BF16)
    nc.sync.dma_start(sinA_c[:, :], sinA_ap)
    mask_c = const.tile([128, 512], BF16)
    nc.sync.dma_start(mask_c[:, :], mask_ap)
    ident_c = const.tile([128, 128], BF16)
    nc.sync.dma_start(ident_c[:, :], ident_ap)

    for bh in range(n_bh):
        qn = big.tile([128, FB], BF16)
        nc.gpsimd.dma_start(
            qn.rearrange("p (n e) -> p n e", e=E),
            q_ap[bh].rearrange("(n p) e -> p n e", p=128),
        )
        kn = big.tile([128, FB], BF16)
        nc.gpsimd.dma_start(
            kn.rearrange("p (n e) -> p n e", e=E),
            k_ap[bh].rearrange("(n p) e -> p n e", p=128),
        )
        vx = big.tile([128, nw * (E + 1)], BF16)
        vx3 = vx.rearrange("p (n c) -> p n c", c=E + 1)
        nc.gpsimd.dma_start(
            vx3[:, :, 0:E], v_ap[bh].rearrange("(n p) e -> p n e", p=128)
        )
        nc.gpsimd.memset(vx3[:, :, E : E + 1], 1.0)

        # rope: x' = x*cos + swapped(x)*sinA (all bf16, DVE 2x mode)
        def rope(xn, name):
            t2 = big.tile([128, FB], BF16, name=f"t2_{name}")
            xb = big.tile([128, FB], BF16, name=f"xb_{name}")
            x4 = xn.rearrange("p (n two h) -> p n two h", two=2, h=HALF)
            t4 = t2.rearrange("p (n two h) -> p n two h", two=2, h=HALF)
            s4 = sinA_c.rearrange("p (n two h) -> p n two h", two=2, h=HALF)
            nc.vector.tensor_mul(t4[:, :, 0, :], x4[:, :, 1, :], s4[:, :, 0, :])
            nc.vector.tensor_mul(t4[:, :, 1, :], x4[:, :, 0, :], s4[:, :, 1, :])
            xc = tbuf.tile([128, FB], BF16, name=f"xc_{name}", tag="xc")
            nc.vector.tensor_mul(xc[:, :], xn[:, :], cos_c[:, :])
            nc.vector.tensor_add(xb[:, :], xc[:, :], t2[:, :])
            return xb

        qrb = rope(qn, "q")
        krb = rope(kn, "k")

        # transposes: one merged PSUM bank per 4 windows (q 0:512, k 512:1024)
        qT = tbuf.tile([64, t_len], BF16)
        kT = tbuf.tile([64, t_len], BF16)
        for g in range(n_grp):
            pt = ptp.tile([64, 2 * WT * 128], BF16)
            for j in range(WT):
                w = g * WT + j
                nc.tensor.matmul(
                    pt[:, j * 128 : (j + 1) * 128],
                    qrb[:, w * E : (w + 1) * E], ident_c[:, :],
                    is_transpose=True,
                )
                nc.tensor.matmul(
                    pt[:, 512 + j * 128 : 512 + (j + 1) * 128],
                    krb[:, w * E : (w + 1) * E], ident_c[:, :],
                    is_transpose=True,
                )
            sl = slice(g * WT * 128, (g + 1) * WT * 128)
            nc.scalar.copy(qT[:, sl], pt[:, 0:512])
            nc.scalar.copy(kT[:, sl], pt[:, 512:1024])

        # score groups of 4 windows / exp / mask / PV / normalize
        out_b = tbuf.tile([128, FB], F32, name="out_b", tag="out_b", bufs=2)
        e_tiles = [None] * n_grp
        for g in range(n_grp):
            st = pst.tile([128, 1024], F32)
            lim = 1024
            for j in range(WT):
                w = g * WT + j
                ncols = 256 if w + 1 < nw else 128
                nc.tensor.matmul(
                    st[:, j * 256 : j * 256 + ncols],
                    kT[:, w * W : (w + 1) * W],
                    qT[:, w * W : w * W + ncols],
                )
                if ncols == 128:
                    lim = j * 256 + 128
            et = ering.tile([128, 1024], BF16)
            nc.scalar.activation(
                et[:, 0:lim], st[:, 0:lim],
                mybir.ActivationFunctionType.Exp, scale=float(E) ** -0.5,
            )
            cur = et.rearrange("p (n c) -> p n c", c=256)[:, :, 0:128]
            msk = mask_c.rearrange("p (n c) -> p n c", c=128)
            nc.vector.tensor_mul(cur, cur, msk)
            e_tiles[g] = et

            ov = pov.tile([128, WT * (E + 1)], F32, name="ov")
            for j in range(WT):
                w = g * WT + j
                osl = slice(j * (E + 1), (j + 1) * (E + 1))
                if w == 0:
                    e_cur = et[:, 0:128]
                    nc.tensor.matmul(
                        ov[:, osl], e_cur, vx3[:, 0, :], start=True, stop=True
                    )
                else:
                    if j == 0:
                        e_prev = e_tiles[g - 1][:, 896:1024]
                    else:
                        e_prev = et[:, (j - 1) * 256 + 128 : j * 256]
                    e_cur = et[:, j * 256 : j * 256 + 128]
                    nc.tensor.matmul(
                        ov[:, osl], e_prev, vx3[:, w - 1, :], start=True, stop=False
                    )
                    nc.tensor.matmul(
                        ov[:, osl], e_cur, vx3[:, w, :], start=False, stop=True
                    )

            ov3 = ov.rearrange("p (n c) -> p n c", c=E + 1)
            r = small.tile([128, WT], F32)
            nc.vector.reciprocal(r[:, :], ov3[:, :, E])
            rb = r[:, :, None].broadcast_to([128, WT, E])
            osl2 = out_b.rearrange("p (n e) -> p n e", e=E)[
                :, g * WT : (g + 1) * WT, :
            ]
            nc.vector.tensor_mul(osl2, ov3[:, :, 0:E], rb)

        nc.sync.dma_start(
            out_ap[bh].rearrange("(n p) e -> p n e", p=128),
            out_b.rearrange("p (n e) -> p n e", e=E),
        )


# ---------------------------------------------------------------- build & run
_CACHE = {}


def _build():
    if "nc" in _CACHE:
        return _CACHE["nc"]
    nc = bacc.Bacc(
        "TRN2",
        target_bir_lowering=False,
        debug=False,
        enable_asserts=True,
        num_devices=N_CORES,
    )
    shp = [N_BH, T, E]
    q = nc.dram_tensor("q", shp, F32, kind="ExternalInput").ap()
    k = nc.dram_tensor("k", shp, F32, kind="ExternalInput").ap()
    v = nc.dram_tensor("v", shp, F32, kind="ExternalInput").ap()
    cos = nc.dram_tensor("cos", [128, NW * E], BF16, kind="ExternalInput").ap()
    sinA = nc.dram_tensor("sinA", [128, NW * E], BF16, kind="ExternalInput").ap()
    mask = nc.dram_tensor("mask", [128, 512], BF16, kind="ExternalInput").ap()
    ident = nc.dram_tensor("ident", [128, 128], BF16, kind="ExternalInput").ap()
    out = nc.dram_tensor("out", shp, F32, kind="ExternalOutput").ap()
    with tile.TileContext(nc) as tc:
        with ExitStack() as ctx:
            _body(ctx, tc, out, q, k, v, cos, sinA, mask, ident, N_BH, NW)
    nc.finalize()
    _CACHE["nc"] = nc
    return nc


def kernel(q, k, v, trace=False):
    nc = _build()
    cos_c, sinA_c = _rope_tables()
    mask = _mask01()
    ident = np.eye(128, dtype=np.float32).astype(NP_BF16)

    qm = np.ascontiguousarray(q.reshape(B * H, T, E))
    km = np.ascontiguousarray(k.reshape(B * H, T, E))
    vm = np.ascontiguousarray(v.reshape(B * H, T, E))
    in_maps = []
    for c in range(N_CORES):
        s = slice(c * N_BH, (c + 1) * N_BH)
        in_maps.append(
            {
                "q": np.ascontiguousarray(qm[s]),
                "k": np.ascontiguousarray(km[s]),
                "v": np.ascontiguousarray(vm[s]),
                "cos": cos_c,
                "sinA": sinA_c,
                "mask": mask,
                "ident": ident,
            }
        )
    res = bass_utils.run_bass_kernel_spmd(
        nc, in_maps, core_ids=list(range(N_CORES)), trace=trace
    )
    out = np.concatenate([r["out"] for r in res.results], axis=0)
    out = out.reshape(q.shape).astype(np.float32)
    if trace:
        return out, res
    return out
